# revision 39
# baseline (speedup 1.0000x reference)
"""Trainium2 Bass kernel for nn_Abstraction (sparse_attention).

Reference computation (per batch element, N=4096, D=512, A=64):
    c      = l2_normalize(data, axis=-1)
    sim    = tril(c @ c.T)                      # [N, N] never materialized here
    pooled = sim.reshape(N, N//A, A).mean(-2)   # [N, A]
    out    = concat([data, pooled @ W_abs], -1) @ W_merge

Key identity used: pooled[n, a] = (1/A) * c_n . ( sum_{g: g*A+a <= n} c_{g*A+a} )
which is a prefix sum over A-strided column groups, so each 64-row block q only
needs the running column-group sums PS_q plus its own diagonal block:
    pooled_block_q = (1/A) * ( C_q @ PS_q^T + tril(C_q @ C_q^T) )

Sharding: 8 cores = 4 batches x 2 row-halves (2048 rows each). Upper-half cores
receive the lower half as an extra input (zeros for lower cores) to compute the
prefix base.  All matmuls run in fp16 operands with fp32 PSUM accumulation.

Schedule notes (engines are in-order; placement = performance):
 - prefix/own loads interleaved 2:1 so the cross-half base beats the chain
 - the serial block-prefix chain starts from zero and is emitted inside the
   per-tile loop so it tracks the load stream on DVE
 - the base contribution goes to a separate PSUM bank (Pb) absorbed in the
   mask stage, so pooled groups close without waiting on the prefix half
 - W_merge is loaded via SWDGE DMA-cast (fp32->fp16 in flight, no cast op)
 - merged output accumulates data-part + pooled-part in one PSUM bank and the
   l2-norm is undone by a per-partition scale in the PSUM->SBUF copy
"""

import os
import sys

sys.path.insert(0, "/opt/trn_rl_repo")

PHASES = int(os.environ.get("K_PHASES", "8"))

import numpy as np

import concourse.bass as bass
import concourse.mybir as mybir
import concourse.tile as tile
from concourse import bacc
from concourse.bass_utils import run_bass_kernel_spmd
from concourse.masks import make_identity

F32 = mybir.dt.float32
F16 = mybir.dt.float16

B, N, D = 4, 4096, 512
A = 64           # abstraction (pool block) size
HALF = N // 2    # rows per core
NT = HALF // 128  # 128-row tiles per core (16)
NB = HALF // A   # 64-row blocks per core (32)
KC = D // 128    # contraction chunks (4)
EPS = 1e-12


def _build_nc():
    nc = bacc.Bacc(None)

    xd = nc.dram_tensor("xd", [HALF, D], F32, kind="ExternalInput")
    xp = nc.dram_tensor("xp", [HALF, D], F32, kind="ExternalInput")
    w_abs = nc.dram_tensor("w_abs", [A, D], F32, kind="ExternalInput")
    w_mrg = nc.dram_tensor("w_mrg", [2 * D, D], F32, kind="ExternalInput")
    out = nc.dram_tensor("out", [HALF, D], F32, kind="ExternalOutput")

    with tile.TileContext(nc) as tc:
        with (
            tc.tile_pool(name="persist", bufs=1) as pp,
            tc.tile_pool(name="load", bufs=6) as lp,
            tc.tile_pool(name="work", bufs=3) as wp,
            tc.tile_pool(name="psum", bufs=2, space="PSUM") as psp,
            tc.tile_pool(name="psum3", bufs=3, space="PSUM") as psp3,
            tc.tile_pool(name="psum1", bufs=1, space="PSUM") as psp1,
            nc.allow_low_precision("fp16 matmul operands by design"),
        ):
            # ---------------- constants ----------------
            ident = pp.tile([128, 128], F16, tag="ident")
            make_identity(nc, ident)

            # mask2: two vertically stacked 64x64 lower-triangular (incl diag)
            # 1.0 masks -> [128, 64]; row p keeps cols a <= (p % 64).
            mask2 = pp.tile([128, A], F32, tag="mask2")
            nc.gpsimd.memset(mask2, 1.0)
            for hh in range(2):
                nc.gpsimd.affine_select(
                    out=mask2[hh * 64:(hh + 1) * 64, :],
                    in_=mask2[hh * 64:(hh + 1) * 64, :],
                    compare_op=mybir.AluOpType.is_ge,
                    fill=0.0,
                    base=0,
                    pattern=[[-1, A]],  # iota = p - a ; keep when >= 0
                    channel_multiplier=1,
                )

            # sel: two stacked 64x64 identities -> [128, 64]; sel[p, a] = (p%64==a)
            sel = pp.tile([128, A], F16, tag="sel")
            nc.gpsimd.memset(sel, 0.0)
            for hh in range(2):
                nc.gpsimd.affine_select(
                    out=sel[hh * 64:(hh + 1) * 64, :],
                    in_=sel[hh * 64:(hh + 1) * 64, :],
                    compare_op=mybir.AluOpType.not_equal,
                    fill=1.0,
                    base=0,
                    pattern=[[-1, A]],
                    channel_multiplier=1,
                )

            eps_sb = pp.tile([128, 1], F32, tag="eps")
            nc.vector.memset(eps_sb, EPS)

            # ---------------- per-row norm state ----------------
            sq = pp.tile([128, NT], F32, tag="sq")        # sum of squares (own)
            norm = pp.tile([128, NT], F32, tag="norm")     # sqrt(sq+eps) (own)
            s_own = pp.tile([128, NT], F32, tag="s_own")   # 1/norm
            s8 = pp.tile([128, NT], F32, tag="s8")         # s/8
            sqp = pp.tile([128, NT], F32, tag="sqp")       # prefix-half variants
            normp = pp.tile([128, NT], F32, tag="normp")
            s_pre = pp.tile([128, NT], F32, tag="s_pre")

            # Xc: transposed normalized data, contiguous columns (lhsT uses)
            Xc = pp.tile([128, KC, HALF], F16, tag="Xc")
            # PSX[ki, k, q, 0:64]   = PS_q (prefix column-group sums), chunk k
            # PSX[ki, k, q, 64:128] = copy of c^T columns of block q (so the
            # pooled matmul's moving operand [PS_q | C_q] is one contiguous AP)
            PSX = pp.tile([128, KC, NB, 2 * A], F16, tag="PSX")

            base_ps = (psp1.tile([A, D], F32, tag="once", name="base_ps")
                       if PHASES >= 3 and not os.environ.get("K_NO_BASEMM") else None)

            # ------- fused load + normalize pipelines (interleaved DMAs) -----
            wabs_st = pp.tile([A, D], F32, tag="wabs_st")

            def load_prefix(t):
                dpt = lp.tile([128, D], F32, tag="dpt", name=f"dpt{t}")
                nc.sync.dma_start(out=dpt, in_=xp[t * 128:(t + 1) * 128, :])
                return dpt

            def load_own(t):
                dt = lp.tile([128, D], F32, tag="dt", name=f"dt{t}")
                nc.sync.dma_start(out=dt, in_=xd[t * 128:(t + 1) * 128, :])
                return dt

            wm16 = pp.tile([128, 2 * KC, D], F16, tag="wm16")
            # W_abs first: it is tiny and gates wcomb (whose psum slot also
            # feeds the Pb base matmuls)
            nc.sync.dma_start(out=wabs_st, in_=w_abs[:, :])
            dpts, dts = {}, {}
            di = iter(range(NT))
            for t in range(NT):
                dpts[t] = load_prefix(t)
                if t % 2 == 1:
                    ti = next(di)
                    dts[ti] = load_own(ti)
            for ti in di:
                dts[ti] = load_own(ti)
            # weights after the data: Wm1 first (merged data-part needs it),
            # then Wm2 + W_abs (feed wcomb).  SWDGE casts fp32->fp16 in
            # flight, so no separate cast op lands on the critical path.
            nc.gpsimd.dma_start(
                out=wm16[:, 0:KC, :],
                in_=w_mrg[0:D, :].rearrange("(ko ki) o -> ki ko o", ki=128),
            )
            nc.gpsimd.dma_start(
                out=wm16[:, KC:2 * KC, :],
                in_=w_mrg[D:2 * D, :].rearrange("(ko ki) o -> ki ko o", ki=128),
            )

            def prefix_tile(t):
                dpt = dpts[t]
                scr2 = wp.tile([128, D], F32, tag="scr2")
                if t % 2 == 0:
                    nc.scalar.activation(
                        out=scr2, in_=dpt,
                        func=mybir.ActivationFunctionType.Square,
                        accum_out=sqp[:, t:t + 1],
                    )
                else:
                    nc.vector.scalar_tensor_tensor(
                        out=scr2, in0=dpt, scalar=1.0, in1=dpt,
                        op0=mybir.AluOpType.mult, op1=mybir.AluOpType.mult,
                        accum_out=sqp[:, t:t + 1],
                    )
                nc.scalar.activation(
                    out=normp[:, t:t + 1], in_=sqp[:, t:t + 1],
                    func=mybir.ActivationFunctionType.Sqrt, bias=eps_sb,
                )
                nc.vector.reciprocal(out=s_pre[:, t:t + 1], in_=normp[:, t:t + 1])
                cp16 = wp.tile([128, D], F16, tag="cp16")
                nc.gpsimd.tensor_scalar_mul(cp16, dpt, s_pre[:, t:t + 1])
                nc.tensor.matmul(
                    base_ps, sel, cp16, start=(t == 0), stop=(t == NT - 1)
                )

            def own_tile(t):
                dt = dts[t]
                scr = wp.tile([128, D], F32, tag="scr")
                nc.scalar.activation(
                    out=scr, in_=dt,
                    func=mybir.ActivationFunctionType.Square,
                    accum_out=sq[:, t:t + 1],
                )
                nc.scalar.activation(
                    out=norm[:, t:t + 1], in_=sq[:, t:t + 1],
                    func=mybir.ActivationFunctionType.Sqrt, bias=eps_sb,
                )
                nc.vector.reciprocal(out=s_own[:, t:t + 1], in_=norm[:, t:t + 1])
                nc.vector.tensor_scalar_mul(
                    s8[:, t:t + 1], s_own[:, t:t + 1], 0.125
                )
                c16 = wp.tile([128, D], F16, tag="c16")
                nc.gpsimd.tensor_scalar_mul(c16, dt, s_own[:, t:t + 1])
                tp = psp1.tile([128, KC, 128], F16, tag="tp")
                for k in range(KC):
                    nc.tensor.transpose(
                        tp[:, k, :], c16[:, k * 128:(k + 1) * 128], ident
                    )
                nc.vector.tensor_copy(out=Xc[:, :, t * 128:(t + 1) * 128], in_=tp)
                nc.gpsimd.tensor_copy(
                    out=PSX[:, :, 2 * t:2 * t + 2, A:2 * A],
                    in_=Xc[:, :, t * 128:(t + 1) * 128].rearrange(
                        "p k (j a) -> p k j a", j=2
                    ),
                )

            def chain_step(q):
                # PS_q = PS_{q-1} + C_{q-1} (prefix over 64-row blocks).
                nc.vector.tensor_tensor(
                    PSX[:, :, q, 0:A],
                    PSX[:, :, q - 1, 0:A],
                    Xc[:, :, (q - 1) * A:q * A],
                    mybir.AluOpType.add,
                )

            # weight prep (off the critical path)
            wabs16 = pp.tile([A, D], F16, tag="wabs16")
            nc.scalar.mul(out=wabs16, in_=wabs_st, mul=0.125)
            wabsT = pp.tile([128, KC, A], F16, tag="wabsT")
            for k in range(KC):
                tps = psp1.tile([128, A], F16, tag="tp")
                nc.tensor.transpose(
                    tps, wabs16[:, k * 128:(k + 1) * 128], ident[0:A, 0:A]
                )
                nc.scalar.copy(out=wabsT[:, k, :], in_=tps)
            wc_ps = psp1.tile([A, D], F32, tag="pb", name="wc_ps")
            for k in range(KC):
                nc.tensor.matmul(
                    wc_ps,
                    wabsT[:, k, :],
                    wm16[:, KC + k, :],
                    start=(k == 0),
                    stop=(k == KC - 1),
                )
            wcomb = pp.tile([A, D], F16, tag="wcomb")
            nc.scalar.copy(out=wcomb, in_=wc_ps)

            # chain starts from zero (local prefix only)
            nc.vector.memset(PSX[:, :, 0, 0:A], 0.0)

            emitted_own = 0

            def own_tile_and_chain():
                nonlocal emitted_own
                t = emitted_own
                own_tile(t)
                for q in (2 * t + 1, 2 * t + 2):
                    if 1 <= q < NB:
                        chain_step(q)
                emitted_own += 1

            for t in range(NT):
                prefix_tile(t)
                if t % 2 == 1:
                    own_tile_and_chain()

            # ---- base: [a, d] psum -> fp16 -> transposed baseT16 [d, a] ----
            base_sb = pp.tile([A, D], F16, tag="base_sb")
            nc.scalar.copy(out=base_sb, in_=base_ps)
            btp = psp1.tile([128, KC, A], F16, tag="tp")
            for k in range(KC):
                nc.tensor.transpose(
                    btp[:, k, :], base_sb[:, k * 128:(k + 1) * 128],
                    ident[0:A, 0:A],
                )
            baseT16 = pp.tile([128, KC, A], F16, tag="baseT16")
            nc.scalar.copy(out=baseT16, in_=btp)

            while emitted_own < NT:
                own_tile_and_chain()



            # ---------------- phase 3: pooled blocks ----------------
            # groups of 4 blocks (= 2 tiles); block q -> psum partitions (q%2)*64
            ps_nat = pp.tile([128, NT, A], F16, tag="ps_nat")
            pps_tiles = {}
            pb_tiles = {}
            for g in range(NB // 4):
                pps = psp3.tile([128, 2, 2 * A], F32, tag="pp", name=f"pps{g}")
                pps_tiles[g] = pps
                for j in range(2):          # tile within group
                    for i in range(2):      # block within tile
                        q = 4 * g + 2 * j + i
                        for k in range(KC):
                            nc.tensor.matmul(
                                pps[i * 64:(i + 1) * 64, j, :],
                                Xc[:, k, q * A:(q + 1) * A],
                                PSX[:, k, q, :],
                                start=(k == 0),
                                stop=(k == KC - 1),
                                tile_position=(0, i * 64),
                            )
                # base contribution c_n . base[a] into its own psum bank so
                # the pps groups close without waiting on the prefix half
                pb = psp1.tile([128, 2, A], F32, tag="pb", name=f"pb{g}")
                pb_tiles[g] = pb
                for j in range(2):
                    t = 2 * g + j
                    for k in range(KC):
                        nc.tensor.matmul(
                            pb[:, j, :],
                            Xc[:, k, t * 128:(t + 1) * 128],
                            baseT16[:, k, :],
                            start=(k == 0),
                            stop=(k == KC - 1),
                        )

            # mask stage emitted after the locals so the DVE priority heap
            # prefers the (critical) chain + Xc copies during the load window
            for g in range(NB // 4):
                pps = pps_tiles[g]
                t1 = wp.tile([128, 2, A], F32, tag="t1")
                nc.vector.tensor_tensor(
                    t1, pps[:, :, A:2 * A],
                    mask2[:, None, :].to_broadcast((128, 2, A)),
                    mybir.AluOpType.mult,
                )
                t2 = wp.tile([128, 2, A], F32, tag="t2")
                nc.vector.tensor_tensor(
                    t2, t1, pps[:, :, 0:A], mybir.AluOpType.add
                )
                t23 = wp.tile([128, 2, A], F32, tag="t23")
                nc.vector.scalar_tensor_tensor(
                    out=t23, in0=pb_tiles[g], scalar=1.0, in1=t2,
                    op0=mybir.AluOpType.mult, op1=mybir.AluOpType.add,
                )
                for j in range(2):
                    t = 2 * g + j
                    nc.vector.tensor_scalar_mul(
                        ps_nat[:, t, :], t23[:, j, :], s8[:, t:t + 1]
                    )

            # transpose pooled to [a, n] for use as merged lhsT
            pooledT = pp.tile([A, NT, 128], F16, tag="pooledT")
            for tg in range(NT // 2):
                ptp = psp1.tile([A, 2, 128], F16, tag="tp")
                for j in range(2):
                    nc.tensor.transpose(
                        ptp[:, j, :], ps_nat[:, 2 * tg + j, :], ident
                    )
                nc.scalar.copy(out=pooledT[:, 2 * tg:2 * tg + 2, :], in_=ptp)

            # ---------------- phase 4: merged output ----------------
            for t in range(NT if PHASES >= 8 else 0):
                mg = psp.tile([128, D], F32, tag="mg")
                for k in range(KC):
                    nc.tensor.matmul(
                        mg,
                        Xc[:, k, t * 128:(t + 1) * 128],
                        wm16[:, k, :],
                        start=(k == 0),
                        stop=False,
                    )
                nc.tensor.matmul(
                    mg, pooledT[:, t, :], wcomb, start=False, stop=True
                )
                osb = wp.tile([128, D], F32, tag="osb")
                nc.scalar.activation(
                    out=osb, in_=mg, func=mybir.ActivationFunctionType.Copy,
                    scale=norm[:, t:t + 1],
                )
                nc.sync.dma_start(out=out[t * 128:(t + 1) * 128, :], in_=osb)

    nc.finalize()
    return nc


_NC_CACHE = None


def _get_nc():
    global _NC_CACHE
    if _NC_CACHE is None:
        _NC_CACHE = _build_nc()
    return _NC_CACHE


_RUNNER = None


def _get_runner():
    """Build (once) a cached jitted SPMD executor for the 8-core kernel."""
    global _RUNNER
    if _RUNNER is not None:
        return _RUNNER

    import jax
    from jax.sharding import Mesh, PartitionSpec
    from jax.experimental.shard_map import shard_map

    import concourse.mybir as mybir
    from concourse import bass2jax

    bass2jax.install_neuronx_cc_hook()
    nc = _get_nc()
    n_cores = 8

    partition_name = (
        nc.partition_id_tensor.name if nc.partition_id_tensor else None
    )
    in_names, out_names, out_shapes, out_dtypes, zero_outs = [], [], [], [], []
    for alloc in nc.m.functions[0].allocations:
        if not isinstance(alloc, mybir.MemoryLocationSet):
            continue
        name = alloc.memorylocations[0].name
        if alloc.kind == "ExternalInput":
            if name != partition_name:
                in_names.append(name)
        elif alloc.kind == "ExternalOutput":
            shape = tuple(alloc.tensor_shape)
            dtype = mybir.dt.np(alloc.dtype)
            out_names.append(name)
            out_shapes.append(shape)
            out_dtypes.append(dtype)
            zero_outs.append(np.zeros(shape, dtype))
    n_params = len(in_names)
    out_avals = [
        jax.core.ShapedArray(s, d) for s, d in zip(out_shapes, out_dtypes)
    ]
    all_in_names = list(in_names) + list(out_names)
    if partition_name is not None:
        all_in_names.append(partition_name)
    donate = tuple(range(n_params, n_params + len(out_names)))

    def _body(*args):
        operands = list(args)
        if partition_name is not None:
            operands.append(bass2jax.partition_id_tensor())
        outs = bass2jax._bass_exec_p.bind(
            *operands,
            out_avals=tuple(out_avals),
            in_names=tuple(all_in_names),
            out_names=tuple(out_names),
            lowering_input_output_aliases=(),
            sim_require_finite=True,
            sim_require_nnan=True,
            nc=nc,
        )
        return tuple(outs)

    devices = jax.devices()[:n_cores]
    mesh = Mesh(np.asarray(devices), ("core",))
    in_specs = (PartitionSpec("core"),) * (n_params + len(out_names))
    out_specs = (PartitionSpec("core"),) * len(out_names)
    sharded = jax.jit(
        shard_map(
            _body, mesh=mesh, in_specs=in_specs, out_specs=out_specs,
            check_rep=False,
        ),
        donate_argnums=donate,
        keep_unused=True,
    )
    _RUNNER = (sharded, in_names, out_names, out_shapes, zero_outs, n_cores)
    return _RUNNER


def _run_fast(in_maps):
    sharded, in_names, out_names, out_shapes, zero_outs, n_cores = _get_runner()
    concat_in = [
        np.concatenate([in_maps[c][nm] for c in range(n_cores)], axis=0)
        for nm in in_names
    ]
    big_zeros = [
        np.zeros((n_cores * z.shape[0],) + z.shape[1:], z.dtype)
        for z in zero_outs
    ]
    out_arrs = sharded(*concat_in, *big_zeros)
    return [
        {
            nm: np.asarray(out_arrs[i]).reshape(
                (n_cores,) + out_shapes[i])[c]
            for i, nm in enumerate(out_names)
        }
        for c in range(n_cores)
    ]


def kernel(data, W_abs, W_merge, _trace=False):
    data = np.ascontiguousarray(np.asarray(data, dtype=np.float32))
    W_abs = np.ascontiguousarray(np.asarray(W_abs, dtype=np.float32))
    W_merge = np.ascontiguousarray(np.asarray(W_merge, dtype=np.float32))
    assert data.shape == (B, N, D)

    zeros_half = np.zeros((HALF, D), np.float32)
    in_maps = []
    for core in range(8):
        b, h = divmod(core, 2)
        in_maps.append({
            "xd": np.ascontiguousarray(data[b, h * HALF:(h + 1) * HALF]),
            "xp": np.ascontiguousarray(data[b, 0:HALF]) if h == 1 else zeros_half,
            "w_abs": W_abs,
            "w_mrg": W_merge,
        })

    if _trace:
        nc = _get_nc()
        res = run_bass_kernel_spmd(
            nc, in_maps, core_ids=list(range(8)), trace=True,
            stitch_traces=True,
        )
        results = res.results
    else:
        res = None
        results = _run_fast(in_maps)

    out = np.empty((B, N, D), np.float32)
    for core in range(8):
        b, h = divmod(core, 2)
        out[b, h * HALF:(h + 1) * HALF] = results[core]["out"]
    if _trace:
        return out, res
    return out


# revision 43
# speedup vs baseline: 1.0001x; 1.0001x over previous
"""Trainium2 Bass kernel for nn_Abstraction (sparse_attention).

Reference computation (per batch element, N=4096, D=512, A=64):
    c      = l2_normalize(data, axis=-1)
    sim    = tril(c @ c.T)                      # [N, N] never materialized here
    pooled = sim.reshape(N, N//A, A).mean(-2)   # [N, A]
    out    = concat([data, pooled @ W_abs], -1) @ W_merge

Key identity used: pooled[n, a] = (1/A) * c_n . ( sum_{g: g*A+a <= n} c_{g*A+a} )
which is a prefix sum over A-strided column groups, so each 64-row block q only
needs the running column-group sums PS_q plus its own diagonal block:
    pooled_block_q = (1/A) * ( C_q @ PS_q^T + tril(C_q @ C_q^T) )

Sharding: 8 cores = 4 batches x 2 row-halves (2048 rows each). Upper-half cores
receive the lower half as an extra input (zeros for lower cores) to compute the
prefix base.  All matmuls run in fp16 operands with fp32 PSUM accumulation.

Schedule notes (engines are in-order; placement = performance):
 - prefix/own loads interleaved 2:1 so the cross-half base beats the chain
 - the serial block-prefix chain starts from zero and is emitted inside the
   per-tile loop so it tracks the load stream on DVE
 - the base contribution goes to a separate PSUM bank (Pb) absorbed in the
   mask stage, so pooled groups close without waiting on the prefix half
 - W_merge is loaded via SWDGE DMA-cast (fp32->fp16 in flight, no cast op)
 - merged output accumulates data-part + pooled-part in one PSUM bank and the
   l2-norm is undone by a per-partition scale in the PSUM->SBUF copy
"""

import os
import sys

sys.path.insert(0, "/opt/trn_rl_repo")

PHASES = int(os.environ.get("K_PHASES", "8"))

import numpy as np

import concourse.bass as bass
import concourse.mybir as mybir
import concourse.tile as tile
from concourse import bacc
from concourse.bass_utils import run_bass_kernel_spmd
from concourse.masks import make_identity

F32 = mybir.dt.float32
F16 = mybir.dt.float16

B, N, D = 4, 4096, 512
A = 64           # abstraction (pool block) size
HALF = N // 2    # rows per core
NT = HALF // 128  # 128-row tiles per core (16)
NB = HALF // A   # 64-row blocks per core (32)
KC = D // 128    # contraction chunks (4)
EPS = 1e-12


def _build_nc():
    nc = bacc.Bacc(None)

    xd = nc.dram_tensor("xd", [HALF, D], F32, kind="ExternalInput")
    xp = nc.dram_tensor("xp", [HALF, D], F32, kind="ExternalInput")
    w_abs = nc.dram_tensor("w_abs", [A, D], F32, kind="ExternalInput")
    w_mrg = nc.dram_tensor("w_mrg", [2 * D, D], F32, kind="ExternalInput")
    out = nc.dram_tensor("out", [HALF, D], F32, kind="ExternalOutput")

    with tile.TileContext(nc) as tc:
        with (
            tc.tile_pool(name="persist", bufs=1) as pp,
            tc.tile_pool(name="load", bufs=6) as lp,
            tc.tile_pool(name="work", bufs=3) as wp,
            tc.tile_pool(name="psum", bufs=2, space="PSUM") as psp,
            tc.tile_pool(name="psum3", bufs=3, space="PSUM") as psp3,
            tc.tile_pool(name="psum1", bufs=1, space="PSUM") as psp1,
            nc.allow_low_precision("fp16 matmul operands by design"),
        ):
            # ---------------- constants ----------------
            ident = pp.tile([128, 128], F16, tag="ident")
            make_identity(nc, ident)

            # mask2: two vertically stacked 64x64 lower-triangular (incl diag)
            # 1.0 masks -> [128, 64]; row p keeps cols a <= (p % 64).
            mask2 = pp.tile([128, A], F32, tag="mask2")
            nc.gpsimd.memset(mask2, 1.0)
            for hh in range(2):
                nc.gpsimd.affine_select(
                    out=mask2[hh * 64:(hh + 1) * 64, :],
                    in_=mask2[hh * 64:(hh + 1) * 64, :],
                    compare_op=mybir.AluOpType.is_ge,
                    fill=0.0,
                    base=0,
                    pattern=[[-1, A]],  # iota = p - a ; keep when >= 0
                    channel_multiplier=1,
                )

            # sel: two stacked 64x64 identities -> [128, 64]; sel[p, a] = (p%64==a)
            sel = pp.tile([128, A], F16, tag="sel")
            nc.gpsimd.memset(sel, 0.0)
            for hh in range(2):
                nc.gpsimd.affine_select(
                    out=sel[hh * 64:(hh + 1) * 64, :],
                    in_=sel[hh * 64:(hh + 1) * 64, :],
                    compare_op=mybir.AluOpType.not_equal,
                    fill=1.0,
                    base=0,
                    pattern=[[-1, A]],
                    channel_multiplier=1,
                )

            eps_sb = pp.tile([128, 1], F32, tag="eps")
            nc.vector.memset(eps_sb, EPS)

            # ---------------- per-row norm state ----------------
            sq = pp.tile([128, NT], F32, tag="sq")        # sum of squares (own)
            norm = pp.tile([128, NT], F32, tag="norm")     # sqrt(sq+eps) (own)
            s_own = pp.tile([128, NT], F32, tag="s_own")   # 1/norm
            s8 = pp.tile([128, NT], F32, tag="s8")         # s/8
            sqp = pp.tile([128, NT], F32, tag="sqp")       # prefix-half variants
            normp = pp.tile([128, NT], F32, tag="normp")
            s_pre = pp.tile([128, NT], F32, tag="s_pre")

            # Xc: transposed normalized data, contiguous columns (lhsT uses)
            Xc = pp.tile([128, KC, HALF], F16, tag="Xc")
            # PSX[ki, k, q, 0:64]   = PS_q (prefix column-group sums), chunk k
            # PSX[ki, k, q, 64:128] = copy of c^T columns of block q (so the
            # pooled matmul's moving operand [PS_q | C_q] is one contiguous AP)
            PSX = pp.tile([128, KC, NB, 2 * A], F16, tag="PSX")

            base_ps = (psp1.tile([A, D], F32, tag="once", name="base_ps")
                       if PHASES >= 3 and not os.environ.get("K_NO_BASEMM") else None)

            # ------- fused load + normalize pipelines (interleaved DMAs) -----
            wabs_st = pp.tile([A, D], F32, tag="wabs_st")

            def load_prefix(t):
                dpt = lp.tile([128, D], F32, tag="dpt", name=f"dpt{t}")
                nc.sync.dma_start(out=dpt, in_=xp[t * 128:(t + 1) * 128, :])
                return dpt

            def load_own(t):
                dt = lp.tile([128, D], F32, tag="dt", name=f"dt{t}")
                nc.sync.dma_start(out=dt, in_=xd[t * 128:(t + 1) * 128, :])
                return dt

            wm16 = pp.tile([128, 2 * KC, D], F16, tag="wm16")
            # W_abs first: it is tiny and gates wcomb (whose psum slot also
            # feeds the Pb base matmuls)
            nc.sync.dma_start(out=wabs_st, in_=w_abs[:, :])
            dpts, dts = {}, {}
            di = iter(range(NT))
            for t in range(NT):
                dpts[t] = load_prefix(t)
                if t % 2 == 1:
                    ti = next(di)
                    dts[ti] = load_own(ti)
            for ti in di:
                dts[ti] = load_own(ti)
            # weights after the data: Wm1 first (merged data-part needs it),
            # then Wm2 + W_abs (feed wcomb).  SWDGE casts fp32->fp16 in
            # flight, so no separate cast op lands on the critical path.
            nc.gpsimd.dma_start(
                out=wm16[:, 0:KC, :],
                in_=w_mrg[0:D, :].rearrange("(ko ki) o -> ki ko o", ki=128),
            )
            nc.gpsimd.dma_start(
                out=wm16[:, KC:2 * KC, :],
                in_=w_mrg[D:2 * D, :].rearrange("(ko ki) o -> ki ko o", ki=128),
            )

            def prefix_tile(t):
                dpt = dpts[t]
                scr2 = wp.tile([128, D], F32, tag="scr2")
                if t % 2 == 0:
                    nc.scalar.activation(
                        out=scr2, in_=dpt,
                        func=mybir.ActivationFunctionType.Square,
                        accum_out=sqp[:, t:t + 1],
                    )
                else:
                    nc.vector.scalar_tensor_tensor(
                        out=scr2, in0=dpt, scalar=1.0, in1=dpt,
                        op0=mybir.AluOpType.mult, op1=mybir.AluOpType.mult,
                        accum_out=sqp[:, t:t + 1],
                    )
                nc.scalar.activation(
                    out=normp[:, t:t + 1], in_=sqp[:, t:t + 1],
                    func=mybir.ActivationFunctionType.Sqrt, bias=eps_sb,
                )
                nc.vector.reciprocal(out=s_pre[:, t:t + 1], in_=normp[:, t:t + 1])
                cp16 = wp.tile([128, D], F16, tag="cp16")
                nc.gpsimd.tensor_scalar_mul(cp16, dpt, s_pre[:, t:t + 1])
                nc.tensor.matmul(
                    base_ps, sel, cp16, start=(t == 0), stop=(t == NT - 1)
                )

            def own_tile(t):
                dt = dts[t]
                scr = wp.tile([128, D], F32, tag="scr")
                nc.scalar.activation(
                    out=scr, in_=dt,
                    func=mybir.ActivationFunctionType.Square,
                    accum_out=sq[:, t:t + 1],
                )
                nc.scalar.activation(
                    out=norm[:, t:t + 1], in_=sq[:, t:t + 1],
                    func=mybir.ActivationFunctionType.Sqrt, bias=eps_sb,
                )
                nc.vector.reciprocal(out=s_own[:, t:t + 1], in_=norm[:, t:t + 1])
                nc.vector.tensor_scalar_mul(
                    s8[:, t:t + 1], s_own[:, t:t + 1], 0.125
                )
                c16 = wp.tile([128, D], F16, tag="c16")
                nc.gpsimd.tensor_scalar_mul(c16, dt, s_own[:, t:t + 1])
                tp = psp1.tile([128, KC, 128], F16, tag="tp")
                for k in range(KC):
                    nc.tensor.transpose(
                        tp[:, k, :], c16[:, k * 128:(k + 1) * 128], ident
                    )
                nc.vector.tensor_copy(out=Xc[:, :, t * 128:(t + 1) * 128], in_=tp)
                nc.gpsimd.tensor_copy(
                    out=PSX[:, :, 2 * t:2 * t + 2, A:2 * A],
                    in_=Xc[:, :, t * 128:(t + 1) * 128].rearrange(
                        "p k (j a) -> p k j a", j=2
                    ),
                )

            def chain_step(q):
                # PS_q = PS_{q-1} + C_{q-1} (prefix over 64-row blocks).
                nc.vector.tensor_tensor(
                    PSX[:, :, q, 0:A],
                    PSX[:, :, q - 1, 0:A],
                    Xc[:, :, (q - 1) * A:q * A],
                    mybir.AluOpType.add,
                )

            # weight prep (off the critical path)
            wabs16 = pp.tile([A, D], F16, tag="wabs16")
            nc.scalar.mul(out=wabs16, in_=wabs_st, mul=0.125)
            wabsT = pp.tile([128, KC, A], F16, tag="wabsT")
            for k in range(KC):
                tps = psp1.tile([128, A], F16, tag="tp")
                nc.tensor.transpose(
                    tps, wabs16[:, k * 128:(k + 1) * 128], ident[0:A, 0:A]
                )
                nc.scalar.copy(out=wabsT[:, k, :], in_=tps)
            wc_ps = psp1.tile([A, D], F32, tag="pb", name="wc_ps")
            for k in range(KC):
                nc.tensor.matmul(
                    wc_ps,
                    wabsT[:, k, :],
                    wm16[:, KC + k, :],
                    start=(k == 0),
                    stop=(k == KC - 1),
                )
            wcomb = pp.tile([A, D], F16, tag="wcomb")
            nc.scalar.copy(out=wcomb, in_=wc_ps)

            # chain starts from zero (local prefix only)
            nc.vector.memset(PSX[:, :, 0, 0:A], 0.0)

            emitted_own = 0

            def own_tile_and_chain():
                nonlocal emitted_own
                t = emitted_own
                own_tile(t)
                for q in (2 * t + 1, 2 * t + 2):
                    if 1 <= q < NB:
                        chain_step(q)
                emitted_own += 1

            for t in range(NT):
                prefix_tile(t)
                if t % 2 == 1:
                    own_tile_and_chain()

            # ---- base: [a, d] psum -> fp16 -> transposed baseT16 [d, a] ----
            base_sb = pp.tile([A, D], F16, tag="base_sb")
            nc.scalar.copy(out=base_sb, in_=base_ps)
            btp = psp1.tile([128, KC, A], F16, tag="tp")
            for k in range(KC):
                nc.tensor.transpose(
                    btp[:, k, :], base_sb[:, k * 128:(k + 1) * 128],
                    ident[0:A, 0:A],
                )
            baseT16 = pp.tile([128, KC, A], F16, tag="baseT16")
            nc.scalar.copy(out=baseT16, in_=btp)

            while emitted_own < NT:
                own_tile_and_chain()




            # ---------------- phase 3: pooled blocks ----------------
            # groups of 4 blocks (= 2 tiles); block q -> psum partitions (q%2)*64
            ps_nat = pp.tile([128, NT, A], F16, tag="ps_nat")
            pps_tiles = {}
            pb_tiles = {}
            for g in range(NB // 4):
                pps = psp.tile([128, 2, 2 * A], F32, tag="pp", name=f"pps{g}")
                pps_tiles[g] = pps
                for j in range(2):          # tile within group
                    for i in range(2):      # block within tile
                        q = 4 * g + 2 * j + i
                        for k in range(KC):
                            nc.tensor.matmul(
                                pps[i * 64:(i + 1) * 64, j, :],
                                Xc[:, k, q * A:(q + 1) * A],
                                PSX[:, k, q, :],
                                start=(k == 0),
                                stop=(k == KC - 1),
                                tile_position=(0, i * 64),
                            )
                # base contribution c_n . base[a] into its own psum bank so
                # the pps groups close without waiting on the prefix half
                pb = psp1.tile([128, 2, A], F32, tag="pb", name=f"pb{g}")
                pb_tiles[g] = pb
                for j in range(2):
                    t = 2 * g + j
                    for k in range(KC):
                        nc.tensor.matmul(
                            pb[:, j, :],
                            Xc[:, k, t * 128:(t + 1) * 128],
                            baseT16[:, k, :],
                            start=(k == 0),
                            stop=(k == KC - 1),
                        )

            # mask stage emitted after the locals so the DVE priority heap
            # prefers the (critical) chain + Xc copies during the load window
            for g in range(NB // 4):
                pps = pps_tiles[g]
                t1 = wp.tile([128, 2, A], F32, tag="t1")
                nc.vector.tensor_tensor(
                    t1, pps[:, :, A:2 * A],
                    mask2[:, None, :].to_broadcast((128, 2, A)),
                    mybir.AluOpType.mult,
                )
                t2 = wp.tile([128, 2, A], F32, tag="t2")
                nc.vector.tensor_tensor(
                    t2, t1, pps[:, :, 0:A], mybir.AluOpType.add
                )
                t23 = wp.tile([128, 2, A], F32, tag="t23")
                nc.vector.scalar_tensor_tensor(
                    out=t23, in0=pb_tiles[g], scalar=1.0, in1=t2,
                    op0=mybir.AluOpType.mult, op1=mybir.AluOpType.add,
                )
                for j in range(2):
                    t = 2 * g + j
                    nc.vector.tensor_scalar_mul(
                        ps_nat[:, t, :], t23[:, j, :], s8[:, t:t + 1]
                    )

            # transpose pooled to [a, n] for use as merged lhsT
            pooledT = pp.tile([A, NT, 128], F16, tag="pooledT")
            for tg in range(NT // 2):
                ptp = psp1.tile([A, 2, 128], F16, tag="tp")
                for j in range(2):
                    nc.tensor.transpose(
                        ptp[:, j, :], ps_nat[:, 2 * tg + j, :], ident
                    )
                nc.scalar.copy(out=pooledT[:, 2 * tg:2 * tg + 2, :], in_=ptp)

            # ---------------- phase 4: merged output ----------------
            for t in range(NT if PHASES >= 8 else 0):
                mg = psp3.tile([128, D], F32, tag="mg")
                for k in range(KC):
                    nc.tensor.matmul(
                        mg,
                        Xc[:, k, t * 128:(t + 1) * 128],
                        wm16[:, k, :],
                        start=(k == 0),
                        stop=False,
                    )
                nc.tensor.matmul(
                    mg, pooledT[:, t, :], wcomb, start=False, stop=True
                )
                osb = wp.tile([128, D], F32, tag="osb")
                nc.scalar.activation(
                    out=osb, in_=mg, func=mybir.ActivationFunctionType.Copy,
                    scale=norm[:, t:t + 1],
                )
                nc.sync.dma_start(out=out[t * 128:(t + 1) * 128, :], in_=osb)

    nc.finalize()
    return nc


_NC_CACHE = None


def _get_nc():
    global _NC_CACHE
    if _NC_CACHE is None:
        _NC_CACHE = _build_nc()
    return _NC_CACHE


_RUNNER = None


def _get_runner():
    """Build (once) a cached jitted SPMD executor for the 8-core kernel."""
    global _RUNNER
    if _RUNNER is not None:
        return _RUNNER

    import jax
    from jax.sharding import Mesh, PartitionSpec
    from jax.experimental.shard_map import shard_map

    import concourse.mybir as mybir
    from concourse import bass2jax

    bass2jax.install_neuronx_cc_hook()
    nc = _get_nc()
    n_cores = 8

    partition_name = (
        nc.partition_id_tensor.name if nc.partition_id_tensor else None
    )
    in_names, out_names, out_shapes, out_dtypes, zero_outs = [], [], [], [], []
    for alloc in nc.m.functions[0].allocations:
        if not isinstance(alloc, mybir.MemoryLocationSet):
            continue
        name = alloc.memorylocations[0].name
        if alloc.kind == "ExternalInput":
            if name != partition_name:
                in_names.append(name)
        elif alloc.kind == "ExternalOutput":
            shape = tuple(alloc.tensor_shape)
            dtype = mybir.dt.np(alloc.dtype)
            out_names.append(name)
            out_shapes.append(shape)
            out_dtypes.append(dtype)
            zero_outs.append(np.zeros(shape, dtype))
    n_params = len(in_names)
    out_avals = [
        jax.core.ShapedArray(s, d) for s, d in zip(out_shapes, out_dtypes)
    ]
    all_in_names = list(in_names) + list(out_names)
    if partition_name is not None:
        all_in_names.append(partition_name)
    donate = tuple(range(n_params, n_params + len(out_names)))

    def _body(*args):
        operands = list(args)
        if partition_name is not None:
            operands.append(bass2jax.partition_id_tensor())
        outs = bass2jax._bass_exec_p.bind(
            *operands,
            out_avals=tuple(out_avals),
            in_names=tuple(all_in_names),
            out_names=tuple(out_names),
            lowering_input_output_aliases=(),
            sim_require_finite=True,
            sim_require_nnan=True,
            nc=nc,
        )
        return tuple(outs)

    devices = jax.devices()[:n_cores]
    mesh = Mesh(np.asarray(devices), ("core",))
    in_specs = (PartitionSpec("core"),) * (n_params + len(out_names))
    out_specs = (PartitionSpec("core"),) * len(out_names)
    sharded = jax.jit(
        shard_map(
            _body, mesh=mesh, in_specs=in_specs, out_specs=out_specs,
            check_rep=False,
        ),
        donate_argnums=donate,
        keep_unused=True,
    )
    _RUNNER = (sharded, in_names, out_names, out_shapes, zero_outs, n_cores)
    return _RUNNER


def _run_fast(in_maps):
    sharded, in_names, out_names, out_shapes, zero_outs, n_cores = _get_runner()
    concat_in = [
        np.concatenate([in_maps[c][nm] for c in range(n_cores)], axis=0)
        for nm in in_names
    ]
    big_zeros = [
        np.zeros((n_cores * z.shape[0],) + z.shape[1:], z.dtype)
        for z in zero_outs
    ]
    out_arrs = sharded(*concat_in, *big_zeros)
    return [
        {
            nm: np.asarray(out_arrs[i]).reshape(
                (n_cores,) + out_shapes[i])[c]
            for i, nm in enumerate(out_names)
        }
        for c in range(n_cores)
    ]


def kernel(data, W_abs, W_merge, _trace=False):
    data = np.ascontiguousarray(np.asarray(data, dtype=np.float32))
    W_abs = np.ascontiguousarray(np.asarray(W_abs, dtype=np.float32))
    W_merge = np.ascontiguousarray(np.asarray(W_merge, dtype=np.float32))
    assert data.shape == (B, N, D)

    zeros_half = np.zeros((HALF, D), np.float32)
    in_maps = []
    for core in range(8):
        b, h = divmod(core, 2)
        in_maps.append({
            "xd": np.ascontiguousarray(data[b, h * HALF:(h + 1) * HALF]),
            "xp": np.ascontiguousarray(data[b, 0:HALF]) if h == 1 else zeros_half,
            "w_abs": W_abs,
            "w_mrg": W_merge,
        })

    if _trace:
        nc = _get_nc()
        res = run_bass_kernel_spmd(
            nc, in_maps, core_ids=list(range(8)), trace=True,
            stitch_traces=True,
        )
        results = res.results
    else:
        res = None
        results = _run_fast(in_maps)

    out = np.empty((B, N, D), np.float32)
    for core in range(8):
        b, h = divmod(core, 2)
        out[b, h * HALF:(h + 1) * HALF] = results[core]["out"]
    if _trace:
        return out, res
    return out
